# revision 48
# baseline (speedup 1.0000x reference)
"""Masked edge attention kernel for 8 Trainium2 NeuronCores.

Reference computation (dims: S=seq=512, B=batch=64, D=dim=512, M=maxlen=512):
    scale[s,b,m] = sum_d M[s,b,d] * W[m,d]
    alpha = softmax(scale, axis=s).transpose(1,2,0)          # (b, m, s)
    mask  = eps everywhere, 1.0 at edges (b,u,v); mask_copy = 0/1 at edges
    scores = (alpha*mask / sum_s(alpha*mask)) * mask_copy

Key algebraic reduction: with X = exp(scale) (no max-subtraction needed,
scale ~ N(0,1)) and Ex = sum_{s in edges} X:
    scores[b,m,s] = mask01[b,m,s] * X[b,m,s] / (eps*T[b,m] + Ex[b,m])
The eps*T term is <= ~1e-5 relative to Ex whenever a row has any edge, and
rows without edges are all-zero anyway, so D = max(Ex, 1e-30) suffices.

Sharding: data-parallel over batch. 8 cores x 8 batches each. W^T replicated.
Host precomputes MT = M.transpose(1,2,0) (d-major per batch), WT = W.T and a
dense uint8 edge mask; device does matmul + exp + fused mask-multiply-reduce
+ scale by reciprocal.
"""

import numpy as np

import concourse.bass as bass
import concourse.mybir as mybir
import concourse.tile as tile
from contextlib import ExitStack

SEQ, BATCH, DIM, MAXLEN = 512, 64, 512, 512
NCORES = 8
BPC = BATCH // NCORES  # batches per core
P = 128
ND = DIM // P      # d chunks
NMI = MAXLEN // P  # m chunks

F32 = mybir.dt.float32
U8 = mybir.dt.uint8


def split_multi_waits(nc):
    """This walrus build accepts at most ONE sync wait per instruction
    ("Too many sync wait commands"), and zero on raw InstISA payloads
    ("ISA wrong length"). Hoist excess waits onto same-engine NoOps
    inserted immediately before the instruction."""
    import bass_rust

    n_new = 0
    for fn in nc.m.functions:
        for blk in fn.blocks:
            out = []
            changed = False
            for inst in blk.instructions:
                keep = 0 if type(inst).__name__ == "InstISA" else 1
                si = inst.sync_info
                ws = list(si.on_wait) if si is not None and si.on_wait else []
                if len(ws) > keep:
                    hoist = ws[: len(ws) - keep]
                    for w in hoist:
                        nop = mybir.InstNoOp(
                            name=f"waitsplit-{n_new}", ins=[], outs=[]
                        )
                        n_new += 1
                        nop.engine = inst.engine
                        nop.sync_info = bass_rust.SyncInfo(
                            on_wait=[w], on_update=[]
                        )
                        out.append(nop)
                    inst.sync_info = bass_rust.SyncInfo(
                        on_wait=ws[len(ws) - keep:],
                        on_update=list(si.on_update) if si.on_update else [],
                    )
                    changed = True
                out.append(inst)
            if changed:
                blk.instructions = out
    return nc


def build_bass(matmul_dtype="float32r", mask_mode="u8_mixed",
               reduce_mode="stt", scale_split="alt", split_waits=True,
               mt_bufs=5, psum_bufs=8, x_bufs=8):
    """matmul_dtype: float32 | float32r | bfloat16
    mask_mode: u8_mixed (tensor ops take u8 mask) | u8_castdma (SWDGE cast)
    reduce_mode: stt (fused scalar_tensor_tensor+accum)
                 | mul_reduce (tensor_mul + reduce_sum)
    scale_split: dve (all final scales on DVE) | alt (alternate DVE/ACT)
                 | dag (spread over DVE/ACT/GPSIMD)
    """
    nc = bass.Bass()
    mm_dt = getattr(mybir.dt, matmul_dtype)
    # float32r is bit-identical to float32 on the host side (np.float32);
    # declaring the tensors f32r end-to-end satisfies walrus's
    # checkMatmultFP32r "rounded" producer rule.
    io_dt = mm_dt if matmul_dtype in ("bfloat16", "float32r") else F32

    # Partition-major DRAM layouts: each SBUF partition's slice is one
    # contiguous 8KB run -> large DMA descriptors (~340 GB/s vs ~290).
    wt = nc.dram_tensor("wt", [P, ND, MAXLEN], io_dt, kind="ExternalInput")
    mt = nc.dram_tensor("mt", [BPC, P, ND, SEQ], io_dt, kind="ExternalInput")
    mask = nc.dram_tensor("mask", [BPC, P, NMI, SEQ], U8, kind="ExternalInput")
    out = nc.dram_tensor("out", [BPC, P, NMI, SEQ], F32, kind="ExternalOutput")

    mask_sb_dt = F32 if mask_mode == "u8_castdma" else U8

    with tile.TileContext(nc) as tc, ExitStack() as ctx:
        singles = ctx.enter_context(tc.tile_pool(name="singles", bufs=1))
        mt_pool = ctx.enter_context(tc.tile_pool(name="mt", bufs=mt_bufs))
        mask_pool = ctx.enter_context(tc.tile_pool(name="mask", bufs=3))
        x_pool = ctx.enter_context(tc.tile_pool(name="x", bufs=x_bufs))
        out_pool = ctx.enter_context(tc.tile_pool(name="outp", bufs=3))
        stat_pool = ctx.enter_context(tc.tile_pool(name="stat", bufs=4))
        psum_pool = ctx.enter_context(
            tc.tile_pool(name="psum", bufs=psum_bufs, space="PSUM")
        )

        # Head critical path: first matmul needs only wt chunk 0 + mt[0]
        # chunk 0 — those two triggers go first, then the bulk loads.
        wt_sb = singles.tile([P, ND, MAXLEN], io_dt)
        mt0_sb = mt_pool.tile([P, ND, SEQ], io_dt, name="mt_sb", tag="mt")
        nc.sync.dma_start(out=wt_sb[:, 0, :], in_=wt[:, 0, :])
        nc.sync.dma_start(out=mt0_sb[:, 0, :], in_=mt[0, :, 0, :])
        nc.sync.dma_start(out=wt_sb[:, 1:, :], in_=wt[:, 1:, :])
        nc.sync.dma_start(out=mt0_sb[:, 1:, :], in_=mt[0, :, 1:, :])

        act_mis = {"dve": [], "alt": [1, 3], "dag": [1, 3]}[scale_split]

        for b in range(BPC):
            if b == 0:
                mt_sb = mt0_sb
            else:
                mt_sb = mt_pool.tile([P, ND, SEQ], io_dt, name="mt_sb",
                                     tag="mt")
                nc.sync.dma_start(out=mt_sb[:], in_=mt[b])
            mask_sb = mask_pool.tile([P, NMI, SEQ], mask_sb_dt)
            if mask_mode == "u8_castdma":
                nc.gpsimd.dma_start(out=mask_sb[:], in_=mask[b])
            else:
                nc.sync.dma_start(out=mask_sb[:], in_=mask[b])
            out_sb = out_pool.tile([P, NMI, SEQ], F32)
            ex_col = stat_pool.tile([P, NMI], F32)
            r_col = stat_pool.tile([P, NMI], F32)

            def mm_tile(ps, mi, di, start, stop):
                lhsT = wt_sb[:, di, mi * P:(mi + 1) * P]
                rhs = mt_sb[:, di, :]
                if mm_dt != io_dt:
                    lhsT = lhsT.bitcast(mm_dt)
                    rhs = rhs.bitcast(mm_dt)
                nc.tensor.matmul(ps[:], lhsT=lhsT, rhs=rhs,
                                 start=start, stop=stop)

            def reduce_tile(ps, mi):
                x_sb = x_pool.tile([P, SEQ], F32, name="x_sb", tag="x")
                nc.scalar.activation(
                    out=x_sb[:], in_=ps[:],
                    func=mybir.ActivationFunctionType.Exp,
                )
                # Y = X*mask into out_sb; Ex = rowsum(Y)
                if reduce_mode == "stt":
                    nc.vector.scalar_tensor_tensor(
                        out=out_sb[:, mi, :], in0=x_sb[:], scalar=1.0,
                        in1=mask_sb[:, mi, :],
                        op0=mybir.AluOpType.mult, op1=mybir.AluOpType.mult,
                        accum_out=ex_col[:, mi:mi + 1],
                    )
                else:
                    nc.vector.tensor_mul(
                        out_sb[:, mi, :], x_sb[:], mask_sb[:, mi, :]
                    )
                    nc.vector.reduce_sum(
                        ex_col[:, mi:mi + 1], out_sb[:, mi, :],
                        axis=mybir.AxisListType.X,
                    )

            def scale_tile(mi, r_ap):
                # R = 1/max(Ex,1e-30): edge rows have Ex >> eps*T; edgeless
                # rows are zeroed by the mask anyway (0 * 1e30 = 0).
                if mi in act_mis:
                    nc.scalar.activation(
                        out=out_sb[:, mi, :], in_=out_sb[:, mi, :],
                        func=mybir.ActivationFunctionType.Copy,
                        scale=r_ap,
                    )
                else:
                    nc.vector.tensor_scalar_mul(
                        out_sb[:, mi, :], out_sb[:, mi, :], r_ap
                    )

            if b == 0:
                # di-major: first matmul waits only on the two chunk-0 loads
                ps_tiles = [
                    psum_pool.tile([P, SEQ], F32, name="ps", tag="ps")
                    for _ in range(NMI)
                ]
                for di in range(ND):
                    for mi in range(NMI):
                        mm_tile(ps_tiles[mi], mi, di, di == 0, di == ND - 1)
                for mi in range(NMI):
                    reduce_tile(ps_tiles[mi], mi)
            else:
                # mi-major: exp/stt of each m-chunk overlaps later matmuls
                for mi in range(NMI):
                    ps = psum_pool.tile([P, SEQ], F32, name="ps", tag="ps")
                    for di in range(ND):
                        mm_tile(ps, mi, di, di == 0, di == ND - 1)
                    reduce_tile(ps, mi)

            if b == BPC - 1:
                # tail: per-mi normalize + write so DMA overlaps last scales
                for mi in range(NMI):
                    nc.vector.tensor_scalar_max(
                        r_col[:, mi:mi + 1], ex_col[:, mi:mi + 1], 1e-30
                    )
                    nc.vector.reciprocal(
                        r_col[:, mi:mi + 1], r_col[:, mi:mi + 1]
                    )
                    scale_tile(mi, r_col[:, mi:mi + 1])
                    # HWDGE here: keeps the SWDGE tail drain off the
                    # critical path (it only covers earlier batches)
                    nc.sync.dma_start(
                        out=out[b, :, mi, :], in_=out_sb[:, mi, :]
                    )
            else:
                nc.vector.tensor_scalar_max(r_col[:], ex_col[:], 1e-30)
                nc.vector.reciprocal(r_col[:], r_col[:])
                for mi in range(NMI):
                    scale_tile(mi, r_col[:, mi:mi + 1])
                nc.gpsimd.dma_start(out=out[b], in_=out_sb[:])
    return split_multi_waits(nc) if split_waits else nc


def prepare_inputs(M, W, edge_b, edge_u, edge_v, io_np_dtype=np.float32):
    M = np.asarray(M, dtype=np.float32)
    W = np.asarray(W, dtype=np.float32)
    # MT[b, p, di, s] = M[s, b, di*128+p]  (partition-major, 8KB runs)
    MT = np.ascontiguousarray(
        M.transpose(1, 2, 0).reshape(BATCH, ND, P, SEQ).transpose(0, 2, 1, 3)
    ).astype(io_np_dtype, copy=False)
    # WT[p, di, m] = W[m, di*128+p]
    WT = np.ascontiguousarray(
        W.T.reshape(ND, P, MAXLEN).transpose(1, 0, 2)
    ).astype(io_np_dtype, copy=False)
    mask8 = np.zeros((BATCH, MAXLEN, SEQ), np.uint8)
    mask8[
        np.asarray(edge_b).astype(np.int64),
        np.asarray(edge_u).astype(np.int64),
        np.asarray(edge_v).astype(np.int64),
    ] = 1
    # mask[b, p, mi, s] = mask8[b, mi*128+p, s]
    mask_t = np.ascontiguousarray(
        mask8.reshape(BATCH, NMI, P, SEQ).transpose(0, 2, 1, 3)
    )
    in_maps = [
        {
            "wt": WT,
            "mt": MT[c * BPC:(c + 1) * BPC],
            "mask": mask_t[c * BPC:(c + 1) * BPC],
        }
        for c in range(NCORES)
    ]
    return in_maps


def unpack_output(core_outs):
    """[BPC, P, NMI, S] per core -> full [BATCH, MAXLEN, SEQ]."""
    full = np.concatenate(core_outs, axis=0)  # [B, P, NMI, S]
    return np.ascontiguousarray(
        full.transpose(0, 2, 1, 3).reshape(BATCH, MAXLEN, SEQ)
    )


_MATMUL_DTYPE = "float32r"
_MASK_MODE = "u8_mixed"
_REDUCE_MODE = "stt"


def kernel(M, W, lengths, edge_b, edge_u, edge_v):
    from concourse.bass_utils import run_bass_kernel_spmd

    io_np = np.float32
    if _MATMUL_DTYPE == "bfloat16":
        import ml_dtypes
        io_np = ml_dtypes.bfloat16
    in_maps = prepare_inputs(M, W, edge_b, edge_u, edge_v, io_np)
    nc = build_bass(_MATMUL_DTYPE, _MASK_MODE, reduce_mode=_REDUCE_MODE)
    res = run_bass_kernel_spmd(nc, in_maps, list(range(NCORES)))
    return unpack_output([res.results[c]["out"] for c in range(NCORES)])
